# revision 14
# baseline (speedup 1.0000x reference)
"""DeepSeekV2 MoE layer on 8 trn2 NeuronCores (expert-parallel).

Strategy (v8):
  - Host: gate softmax + group-limited top-k routing -> per-expert sorted token
    lists and combine weights (control data only; all heavy FLOPs on device).
  - Experts are rank-matched to (core, slot): sort by token count desc, slot j
    holds ranks [8j, 8j+8) so slot capacity = count of its largest expert.
    This trims ~7-10% of the padded matmul rows vs one global CAP.
  - Device (SPMD over 8 cores, 4 expert slots each):
      Routed phase: per slot, transposed dma_gathers of its CAP_j tokens in
      chunks [256, 512, ...] (small first chunk -> first matmul starts early)
      on the single Tile-managed swdge queue (multi-queue swdge is racy: sem
      assignment is queue-unaware); mm1/mm3 (fp16) -> silu*mul -> mm2 ->
      scale by combine weight; one scatter-add per (expert, 512-col block)
      into the dense y[T+128, H] (pad entries -> row T).
      Weight loads (w13/w2) are split in half across the two hwdge queues
      (sync + scalar) via separate half-tiles - a single queue can't sustain
      the stream and stalls the PE; two DMAs into one tile is a race (Tile
      dep-tracking is tile-granular).
      y zeroing is 32 SBUF->DRAM writes from a memset tile emitted after
      expert 0's mm13 issue (write-only; off the early weight-load window).
      A 2MB warmup ReduceScatter (garbage in, discarded out) after the first
      gather + small per-expert heartbeat RS for e=0..2 keep the CC fabric
      clocked up: collectives here cost ~75-100us nearly independent of size,
      and a cold fabric ran an 0.25MB RS at ~2.5GB/s.
      ONE ReduceScatter over y[0:T] (not two over row-halves: each extra
      collective pays the ~75us fixed cost) -> rs[512, H]; core c's own
      tokens are the contiguous block [512c, 512c+512).
      Shared experts (full SI) for this core's own 512 rows overlap the RS;
      strict queue discipline in the shared phase: s13/s2 loads on scalar
      ONLY, rs reads + out stores on sync ONLY (an RS-gated kick ahead of a
      compute load on an in-order queue stalls the PE for the whole RS).
  - Host: reassemble contiguous 512-row blocks -> [B, S, H].
"""
import sys

import numpy as np

sys.path.insert(0, "/opt/trn_rl_repo")

import concourse.bass as bass
import concourse.mybir as mybir
import concourse.tile as tile
from concourse import bacc
from concourse.bass_utils import run_bass_kernel_spmd

F32 = mybir.dt.float32
FP16 = mybir.dt.float16
I16 = mybir.dt.int16
AF = mybir.ActivationFunctionType
OP = mybir.AluOpType

N_GROUP, TOPK_GROUP, TOP_K = 8, 3, 6
NCORES = 8


def _routing(x, gate_w):
    T, E = x.shape[0], gate_w.shape[0]
    logits = (x @ gate_w.T).astype(np.float64)
    e = np.exp(logits - logits.max(-1, keepdims=True))
    scores = e / e.sum(-1, keepdims=True)
    per_group = E // N_GROUP
    group_scores = scores.reshape(T, N_GROUP, per_group).max(-1)
    order = np.argsort(-group_scores, axis=-1, kind="stable")
    group_mask = np.zeros((T, N_GROUP), bool)
    np.put_along_axis(group_mask, order[:, :TOPK_GROUP], True, axis=1)
    tmp = np.where(np.repeat(group_mask, per_group, axis=1), scores, 0.0)
    order_e = np.argsort(-tmp, axis=-1, kind="stable")
    topk_idx = order_e[:, :TOP_K]
    topk_w = np.take_along_axis(tmp, topk_idx, axis=1)
    topk_w = topk_w / (topk_w.sum(-1, keepdims=True) + 1e-20)
    combine = np.zeros((T, E), np.float32)
    np.put_along_axis(combine, topk_idx, topk_w.astype(np.float32), axis=1)
    return combine


def _wrap16(a):
    """[n] int16 -> [128, n//16] index layout for dma_gather/scatter."""
    return np.tile(a.reshape(-1, 16).T, (8, 1))


def _chunks(cap):
    """psum-chain column widths: [512, ..., remainder]."""
    out = []
    rem = cap
    while rem >= 512:
        out.append(512)
        rem -= 512
    if rem:
        out.append(rem)
    return out


def build_kernel(T, H, I, EPC, CAPS, SI, act=AF.Silu, compile_=True):
    KT = H // 128          # contraction tiles over H
    MT = I // 128          # I tiles
    SIT = SI // 128
    TOUT = T // NCORES     # own output rows
    TS = TOUT // 128
    NSTR = H // 512
    CAPM = max(CAPS)
    CTM = CAPM // 128
    CHUNKS = [_chunks(c) for c in CAPS]

    nc = bacc.Bacc("TRN2")
    xgd = nc.dram_tensor("xgd", [EPC, 128, KT * CAPM], FP16, kind="ExternalInput")
    xTc = nc.dram_tensor("xTc", [128, KT * TOUT], FP16, kind="ExternalInput")
    w13 = nc.dram_tensor("w13", [EPC, MT, 128, KT * 256], FP16, kind="ExternalInput")
    w2b = nc.dram_tensor("w2b", [EPC, 4, 128, MT * 512], FP16, kind="ExternalInput")
    sw13 = nc.dram_tensor("sw13", [SIT, 128, KT * 256], FP16, kind="ExternalInput")
    sw2b = nc.dram_tensor("sw2b", [2 * NSTR, 128, SIT * 256], FP16, kind="ExternalInput")
    idxs = nc.dram_tensor("idxs", [EPC, 128, CAPM // 16], I16, kind="ExternalInput")
    gat = nc.dram_tensor("gat", [EPC, 128, CTM], F32, kind="ExternalInput")
    out = nc.dram_tensor("out", [TOUT, H], FP16, kind="ExternalOutput")

    yL = nc.dram_tensor("yL", [T + 128, H // 2], FP16)
    yR = nc.dram_tensor("yR", [T + 128, H // 2], FP16)
    rsL = nc.dram_tensor("rsL", [TOUT, H // 2], FP16)
    rsR = nc.dram_tensor("rsR", [TOUT, H // 2], FP16)
    warm_in = nc.dram_tensor("warm_in", [2048, 512], FP16)
    warm_out = nc.dram_tensor("warm_out", [256, 512], FP16)
    hb_out = [nc.dram_tensor(f"hb_out{e}", [32, 512], FP16) for e in range(EPC)]

    grp = [list(range(NCORES))]

    with tile.TileContext(nc) as tc:
        with (
            tc.tile_pool(name="const", bufs=1) as const,
            tc.tile_pool(name="persist", bufs=1) as persist,
            tc.tile_pool(name="xgtp", bufs=2) as xgtp,
            tc.tile_pool(name="xgtp1", bufs=2) as xgtp1,
            tc.tile_pool(name="gp", bufs=2) as gp,
            tc.tile_pool(name="w13p", bufs=2) as w13p,
            tc.tile_pool(name="w2p", bufs=2) as w2p,
            tc.tile_pool(name="ybp", bufs=2) as ybp,
            tc.tile_pool(name="s13p", bufs=2) as s13p,
            tc.tile_pool(name="s2p", bufs=2) as s2p,
            tc.tile_pool(name="small", bufs=2) as small,
            tc.tile_pool(name="rsp", bufs=1) as rsp,
            tc.tile_pool(name="psum", bufs=2, space="PSUM") as psum,
        ):
            # ---------------- constants ------------------------------------
            isc = const.tile([128, EPC, CAPM // 16], I16)
            nc.sync.dma_start(isc[:], idxs.rearrange("e p c -> p e c"))
            ga_sb = const.tile([128, EPC, CTM], F32)
            nc.sync.dma_start(ga_sb[:], gat.rearrange("e p c -> p e c"))
            zt = const.tile([128, 1024], FP16)
            nc.vector.memset(zt[:], 0.0)

            gs = persist.tile([128, SIT, TOUT], FP16)
            xtc_sb = persist.tile([128, KT, TOUT], FP16)

            # ---------------- routed experts -------------------------------
            for e in range(EPC):
                CH = CHUNKS[e]
                CAP = CAPS[e]
                CT = CAP // 128
                # token activations pre-gathered+transposed on the host:
                # plain contiguous loads on the fast hwdge queues (the
                # device-side transposed gather was a ~100us startup
                # staircase and hogged the single swdge queue)
                xgta = xgtp.tile([128, KT // 2, CAPM], FP16, tag="xgta")
                xgtb = xgtp1.tile([128, KT // 2, CAPM], FP16, tag="xgtb")
                xgs = xgd[e].rearrange("p (k c) -> p k c", c=CAPM)
                nc.sync.dma_start(xgta[:], xgs[:, :KT // 2, :])
                nc.scalar.dma_start(xgtb[:], xgs[:, KT // 2:, :])
                if e == 0:
                    # warmup collective: after the first gather so it doesn't
                    # delay it (transpose DMAs serialize with collectives).
                    # warm_in is uninitialized garbage - result is discarded.
                    nc.gpsimd.collective_compute(
                        "ReduceScatter", OP.add, replica_groups=grp,
                        ins=[warm_in[:]], outs=[warm_out[:]])
                g = gp.tile([128, MT, CAPM], FP16, tag="g")
                for m in range(MT):
                    w13ta = w13p.tile([128, KT // 2, 256], FP16, tag="w13ta")
                    w13tb = w13p.tile([128, KT // 2, 256], FP16, tag="w13tb")
                    w13s = w13[e, m].rearrange("p (k c) -> p k c", c=256)
                    nc.sync.dma_start(w13ta[:], w13s[:, :KT // 2, :])
                    nc.scalar.dma_start(w13tb[:], w13s[:, KT // 2:, :])
                    c0 = 0
                    for ci, cw in enumerate(CH):
                        p1 = psum.tile([128, 512], F32, tag="p1")
                        p3 = psum.tile([128, 512], F32, tag="p3")
                        for k in range(KT):
                            wh = w13ta if k < KT // 2 else w13tb
                            xh = xgta if k < KT // 2 else xgtb
                            nc.tensor.matmul(p1[:, :cw], wh[:, k % (KT // 2), :128],
                                             xh[:, k % (KT // 2), c0:c0 + cw],
                                             start=(k == 0), stop=(k == KT - 1))
                        for k in range(KT):
                            wh = w13ta if k < KT // 2 else w13tb
                            xh = xgta if k < KT // 2 else xgtb
                            nc.tensor.matmul(p3[:, :cw], wh[:, k % (KT // 2), 128:],
                                             xh[:, k % (KT // 2), c0:c0 + cw],
                                             start=(k == 0), stop=(k == KT - 1))
                        nc.scalar.activation(g[:, m, c0:c0 + cw], p1[:, :cw], act)
                        nc.vector.tensor_tensor(g[:, m, c0:c0 + cw],
                                                g[:, m, c0:c0 + cw],
                                                p3[:, :cw], OP.mult)
                        c0 += cw
                if e == 0:
                    # zero y from the SBUF memset tile (write-only HBM
                    # traffic). yL halves ride sync+scalar after e0's w13
                    # kicks; yR rides the swdge queue, which is idle until
                    # the first scatter (~160us) now that gathers are
                    # host-side - its q2/q3 scatters queue behind naturally.
                    for r in range(0, T, 256):
                        nc.sync.dma_start(yL[r:r + 128, :], zt[:])
                        nc.scalar.dma_start(yL[r + 128:r + 256, :], zt[:])
                    for r in range(0, T, 128):
                        nc.gpsimd.dma_start(yR[r:r + 128, :], zt[:])
                if e == 2:
                    # shared-expert input tokens (needed only at the end;
                    # emitted here to keep it off the startup-critical queues)
                    nc.scalar.dma_start(
                        xtc_sb[:], xTc.rearrange("p (k t) -> p k t", t=TOUT))
                if e < EPC - 1:
                    # zero-dependency heartbeat collective (garbage slice of
                    # warm_in, result discarded): keeps the CC fabric clocked
                    # up through the routed phase (cold-fabric RS runs
                    # ~10-30x slower). No data deps -> Tile cannot entangle
                    # it with compute streams; it launches when the gpsimd
                    # stream reaches it, naturally spaced per expert. None
                    # for the last expert - it would delay the big RS.
                    nc.gpsimd.collective_compute(
                        "ReduceScatter", OP.add, replica_groups=grp,
                        ins=[warm_in[256 * (e + 1):256 * (e + 2), :]],
                        outs=[hb_out[e][:]])
                for q in range(4):
                    w2ta = w2p.tile([128, MT // 2, 512], FP16, tag="w2ta")
                    w2tb = w2p.tile([128, MT // 2, 512], FP16, tag="w2tb")
                    w2s = w2b[e, q].rearrange("p (k c) -> p k c", c=512)
                    nc.sync.dma_start(w2ta[:], w2s[:, :MT // 2, :])
                    nc.scalar.dma_start(w2tb[:], w2s[:, MT // 2:, :])
                    yb = ybp.tile([128, CTM, 512], FP16, tag="yb")
                    for ct in range(CT):
                        p4a = psum.tile([128, 512], F32, tag="p4a")
                        for k2 in range(MT):
                            w2h = w2ta if k2 < MT // 2 else w2tb
                            nc.tensor.matmul(p4a[:], g[:, k2, ct * 128:(ct + 1) * 128],
                                             w2h[:, k2 % (MT // 2), :],
                                             start=(k2 == 0), stop=(k2 == MT - 1))
                        nc.vector.tensor_tensor(
                            yb[:, ct, :], p4a[:],
                            ga_sb[:, e, ct:ct + 1].to_broadcast([128, 512]),
                            OP.mult)
                    # one scatter-add per 512-col block (pad entries -> row
                    # T); cols 0:1024 -> yL, 1024:2048 -> yR so RS_L can
                    # launch as soon as every expert's q=0,1 scatters land
                    ydst = yL if q < 2 else yR
                    nc.gpsimd.dma_scatter_add(
                        ydst[:, (q % 2) * 512:(q % 2 + 1) * 512], yb[:, :CT, :],
                        isc[:, e, :CAP // 16], CAP, CAP, 512, elem_step=H // 2)

            # ---------------- shared experts (own rows) --------------------
            # gs is emitted BEFORE the RS issue: program order guarantees no
            # engine stream has an RS-gated entry ahead of the s13 loads, so
            # the gs compute overlaps the collective.
            for sm in range(SIT):
                s13a = s13p.tile([128, KT // 2, 256], FP16, tag="s13a")
                s13b = s13p.tile([128, KT // 2, 256], FP16, tag="s13b")
                s13s = sw13[sm].rearrange("p (k c) -> p k c", c=256)
                nc.sync.dma_start(s13a[:], s13s[:, :KT // 2, :])
                nc.scalar.dma_start(s13b[:], s13s[:, KT // 2:, :])
                p1 = psum.tile([128, 512], F32, tag="p1")
                p3 = psum.tile([128, 512], F32, tag="p3")
                for k in range(KT):
                    sh = s13a if k < KT // 2 else s13b
                    nc.tensor.matmul(p1[:, :TOUT], sh[:, k % (KT // 2), :128],
                                     xtc_sb[:, k, :],
                                     start=(k == 0), stop=(k == KT - 1))
                for k in range(KT):
                    sh = s13a if k < KT // 2 else s13b
                    nc.tensor.matmul(p3[:, :TOUT], sh[:, k % (KT // 2), 128:],
                                     xtc_sb[:, k, :],
                                     start=(k == 0), stop=(k == KT - 1))
                nc.scalar.activation(gs[:, sm, :], p1[:, :TOUT], act)
                nc.vector.tensor_tensor(gs[:, sm, :], gs[:, sm, :], p3[:, :TOUT],
                                        OP.mult)

            nc.gpsimd.collective_compute(
                "ReduceScatter", OP.add, replica_groups=grp,
                ins=[yL[0:T, :]], outs=[rsL[:]])
            nc.gpsimd.collective_compute(
                "ReduceScatter", OP.add, replica_groups=grp,
                ins=[yR[0:T, :]], outs=[rsR[:]])
            # whole rs halves -> SBUF via the (now idle) swdge queue: two big
            # RS-gated loads there instead of 64 small ones on sync, which
            # wedged the sync queue for ~100us behind the collectives
            rsl_sb = rsp.tile([128, TS, H // 2], FP16, tag="rsb")
            nc.gpsimd.dma_start(rsl_sb[:], rsL.rearrange("(t p) w -> p t w", p=128))
            rsr_sb = rsp.tile([128, TS, H // 2], FP16, tag="rsb")
            nc.gpsimd.dma_start(rsr_sb[:], rsR.rearrange("(t p) w -> p t w", p=128))

            # shared out per 256-col strip + combine with rs.
            # Queue discipline: s2 loads on scalar ONLY; rs reads + out
            # stores on sync ONLY (sync carries nothing compute-path here,
            # so RS-gated kicks can't stall the shared weight stream).
            for s in range(2 * NSTR):
                s2a = s2p.tile([128, SIT // 2, 256], FP16, tag="s2a")
                s2b = s2p.tile([128, SIT // 2, 256], FP16, tag="s2b")
                s2s = sw2b[s].rearrange("p (k c) -> p k c", c=256)
                nc.sync.dma_start(s2a[:], s2s[:, :SIT // 2, :])
                nc.scalar.dma_start(s2b[:], s2s[:, SIT // 2:, :])
                rsh_sb = rsl_sb if s < NSTR else rsr_sb
                sc0 = (s % NSTR) * 256
                for ts in range(TS):
                    po = psum.tile([128, 256], F32, tag="p4a")
                    for k2 in range(SIT):
                        s2h = s2a if k2 < SIT // 2 else s2b
                        nc.tensor.matmul(po[:], gs[:, k2, ts * 128:(ts + 1) * 128],
                                         s2h[:, k2 % (SIT // 2), :],
                                         start=(k2 == 0), stop=(k2 == SIT - 1))
                    ott = small.tile([128, 256], FP16, tag="ott")
                    nc.vector.tensor_tensor(ott[:], po[:],
                                            rsh_sb[:, ts, sc0:sc0 + 256], OP.add)
                    nc.sync.dma_start(
                        out[ts * 128:(ts + 1) * 128, s * 256:(s + 1) * 256],
                        ott[:])

    if compile_:
        nc.compile()
    else:
        nc.insert_library_loads()
    return nc


def host_prep(hidden_states, gate_weight, w1, w2, w3, sw1, sw2, sw3):
    B, S, H = hidden_states.shape
    T = B * S
    E, I = w1.shape[0], w1.shape[1]
    SI = sw1.shape[0]
    EPC = E // NCORES
    KT, MT, SIT = H // 128, I // 128, SI // 128
    NSTR = H // 512
    TOUT = T // NCORES

    x = np.ascontiguousarray(hidden_states.reshape(T, H), dtype=np.float32)
    combine = _routing(x, gate_weight.astype(np.float32))
    tok_lists = [np.nonzero(combine[:, e])[0] for e in range(E)]
    counts = np.array([len(t) for t in tok_lists])

    # rank-matched expert assignment: sort by count desc; slot j holds ranks
    # [8j, 8j+8); core c gets order[8j + c]. Slot capacity covers its max.
    order = np.argsort(-counts, kind="stable")
    CAPS = [max(256, int(np.ceil(counts[order[8 * j]] / 128) * 128))
            for j in range(EPC)]
    CAPM = max(CAPS)
    CTM = CAPM // 128

    x16 = x.astype(np.float16)
    xT = x.T  # [H, T] view

    s1 = sw1.T.reshape(KT, 128, SIT, 128).transpose(2, 1, 0, 3)
    s3 = sw3.T.reshape(KT, 128, SIT, 128).transpose(2, 1, 0, 3)
    sw13 = np.ascontiguousarray(
        np.concatenate([s1, s3], axis=-1).reshape(SIT, 128, -1), dtype=np.float16)
    sw2b = np.ascontiguousarray(
        sw2.T.reshape(SIT, 128, 2 * NSTR, 256).transpose(2, 1, 0, 3)
        .reshape(2 * NSTR, 128, -1), dtype=np.float16)

    xt16 = np.ascontiguousarray(xT, dtype=np.float16)  # [H, T]
    in_maps = []
    for c in range(NCORES):
        els = [int(order[8 * j + c]) for j in range(EPC)]
        xgd = np.zeros((EPC, 128, KT * CAPM), np.float16)
        idxs = np.zeros((EPC, 128, CAPM // 16), np.int16)
        gatv = np.zeros((EPC, 128, CTM), np.float32)
        for j, e in enumerate(els):
            CAP = CAPS[j]
            toks = tok_lists[e]
            n = len(toks)
            # pre-gathered transposed tokens: xgd[j][p, k*CAPM+c] =
            # x16[toks[c], k*128+p] (pads -> 0; their gate weight is 0)
            sel = np.zeros((H, CAPM), np.float16)
            sel[:, :n] = xt16[:, toks]
            xgd[j] = (sel.reshape(KT, 128, CAPM).transpose(1, 0, 2)
                      .reshape(128, -1))
            # scatter: valid entries keep their token row, pads -> row T
            sc = np.full(CAP, T, np.int16)
            sc[:n] = toks
            idxs[j, :, :CAP // 16] = _wrap16(sc)
            gv = np.zeros(CAP, np.float32)
            gv[:n] = combine[toks, e]
            gatv[j, :, :CAP // 128] = gv.reshape(-1, 128).T
        w13c = np.empty((EPC, MT, 128, KT * 256), np.float16)
        w2c = np.empty((EPC, 4, 128, MT * 512), np.float16)
        for j, e in enumerate(els):
            a1 = w1[e].T.reshape(KT, 128, MT, 128).transpose(2, 1, 0, 3)
            a3 = w3[e].T.reshape(KT, 128, MT, 128).transpose(2, 1, 0, 3)
            w13c[j] = np.concatenate([a1, a3], axis=-1).reshape(MT, 128, -1)
            w2c[j] = (w2[e].T.reshape(MT, 128, 4, 512)
                      .transpose(2, 1, 0, 3).reshape(4, 128, -1))
        own_rows = np.arange(c * TOUT, (c + 1) * TOUT)
        xTc = np.ascontiguousarray(
            xT[:, own_rows].reshape(KT, 128, len(own_rows))
            .transpose(1, 0, 2).reshape(128, -1), dtype=np.float16)
        in_maps.append({
            "xgd": xgd, "xTc": xTc,
            "w13": w13c, "w2b": w2c,
            "sw13": sw13, "sw2b": sw2b,
            "idxs": idxs,
            "gat": gatv,
        })
    cfg = dict(T=T, H=H, I=I, EPC=EPC, CAPS=CAPS, SI=SI)
    return in_maps, cfg


def kernel(**inputs):
    inputs = {k: np.asarray(v) for k, v in inputs.items()}
    hs = inputs["hidden_states"]
    B, S, H = hs.shape
    in_maps, cfg = host_prep(
        hs, inputs["gate_weight"], inputs["w1"], inputs["w2"], inputs["w3"],
        inputs["sw1"], inputs["sw2"], inputs["sw3"])
    nc = build_kernel(**cfg)
    res = run_bass_kernel_spmd(nc, in_maps, list(range(NCORES)))
    T = B * S
    TOUT = T // NCORES
    y = np.empty((T, H), np.float32)
    for c in range(NCORES):
        y[c * TOUT:(c + 1) * TOUT] = res.results[c]["out"]
    return y.reshape(B, S, H).astype(np.float32)


if __name__ == "__main__":
    pass


# revision 15
# speedup vs baseline: 1.0115x; 1.0115x over previous
"""DeepSeekV2 MoE layer on 8 trn2 NeuronCores (expert-parallel).

Strategy (v8):
  - Host: gate softmax + group-limited top-k routing -> per-expert sorted token
    lists and combine weights (control data only; all heavy FLOPs on device).
  - Experts are rank-matched to (core, slot): sort by token count desc, slot j
    holds ranks [8j, 8j+8) so slot capacity = count of its largest expert.
    This trims ~7-10% of the padded matmul rows vs one global CAP.
  - Device (SPMD over 8 cores, 4 expert slots each):
      Routed phase: per slot, transposed dma_gathers of its CAP_j tokens in
      chunks [256, 512, ...] (small first chunk -> first matmul starts early)
      on the single Tile-managed swdge queue (multi-queue swdge is racy: sem
      assignment is queue-unaware); mm1/mm3 (fp16) -> silu*mul -> mm2 ->
      scale by combine weight; one scatter-add per (expert, 512-col block)
      into the dense y[T+128, H] (pad entries -> row T).
      Weight loads (w13/w2) are split in half across the two hwdge queues
      (sync + scalar) via separate half-tiles - a single queue can't sustain
      the stream and stalls the PE; two DMAs into one tile is a race (Tile
      dep-tracking is tile-granular).
      y zeroing is 32 SBUF->DRAM writes from a memset tile emitted after
      expert 0's mm13 issue (write-only; off the early weight-load window).
      A 2MB warmup ReduceScatter (garbage in, discarded out) after the first
      gather + small per-expert heartbeat RS for e=0..2 keep the CC fabric
      clocked up: collectives here cost ~75-100us nearly independent of size,
      and a cold fabric ran an 0.25MB RS at ~2.5GB/s.
      ONE ReduceScatter over y[0:T] (not two over row-halves: each extra
      collective pays the ~75us fixed cost) -> rs[512, H]; core c's own
      tokens are the contiguous block [512c, 512c+512).
      Shared experts (full SI) for this core's own 512 rows overlap the RS;
      strict queue discipline in the shared phase: s13/s2 loads on scalar
      ONLY, rs reads + out stores on sync ONLY (an RS-gated kick ahead of a
      compute load on an in-order queue stalls the PE for the whole RS).
  - Host: reassemble contiguous 512-row blocks -> [B, S, H].
"""
import sys

import numpy as np

sys.path.insert(0, "/opt/trn_rl_repo")

import concourse.bass as bass
import concourse.mybir as mybir
import concourse.tile as tile
from concourse import bacc
from concourse.bass_utils import run_bass_kernel_spmd

F32 = mybir.dt.float32
FP16 = mybir.dt.float16
I16 = mybir.dt.int16
AF = mybir.ActivationFunctionType
OP = mybir.AluOpType

N_GROUP, TOPK_GROUP, TOP_K = 8, 3, 6
NCORES = 8


def _routing(x, gate_w):
    T, E = x.shape[0], gate_w.shape[0]
    logits = (x @ gate_w.T).astype(np.float64)
    e = np.exp(logits - logits.max(-1, keepdims=True))
    scores = e / e.sum(-1, keepdims=True)
    per_group = E // N_GROUP
    group_scores = scores.reshape(T, N_GROUP, per_group).max(-1)
    order = np.argsort(-group_scores, axis=-1, kind="stable")
    group_mask = np.zeros((T, N_GROUP), bool)
    np.put_along_axis(group_mask, order[:, :TOPK_GROUP], True, axis=1)
    tmp = np.where(np.repeat(group_mask, per_group, axis=1), scores, 0.0)
    order_e = np.argsort(-tmp, axis=-1, kind="stable")
    topk_idx = order_e[:, :TOP_K]
    topk_w = np.take_along_axis(tmp, topk_idx, axis=1)
    topk_w = topk_w / (topk_w.sum(-1, keepdims=True) + 1e-20)
    combine = np.zeros((T, E), np.float32)
    np.put_along_axis(combine, topk_idx, topk_w.astype(np.float32), axis=1)
    return combine


def _wrap16(a):
    """[n] int16 -> [128, n//16] index layout for dma_gather/scatter."""
    return np.tile(a.reshape(-1, 16).T, (8, 1))


def _chunks(cap):
    """psum-chain column widths: [512, ..., remainder]."""
    out = []
    rem = cap
    while rem >= 512:
        out.append(512)
        rem -= 512
    if rem:
        out.append(rem)
    return out


def build_kernel(T, H, I, EPC, CAPS, SI, act=AF.Silu, compile_=True):
    KT = H // 128          # contraction tiles over H
    MT = I // 128          # I tiles
    SIT = SI // 128
    TOUT = T // NCORES     # own output rows
    TS = TOUT // 128
    NSTR = H // 512
    CAPM = max(CAPS)
    CTM = CAPM // 128
    CHUNKS = [_chunks(c) for c in CAPS]

    nc = bacc.Bacc("TRN2")
    xgd = nc.dram_tensor("xgd", [EPC, 128, KT * CAPM], FP16, kind="ExternalInput")
    xTc = nc.dram_tensor("xTc", [128, KT * TOUT], FP16, kind="ExternalInput")
    w13 = nc.dram_tensor("w13", [EPC, MT, 128, KT * 256], FP16, kind="ExternalInput")
    w2b = nc.dram_tensor("w2b", [EPC, 4, 128, MT * 512], FP16, kind="ExternalInput")
    sw13 = nc.dram_tensor("sw13", [SIT, 128, KT * 256], FP16, kind="ExternalInput")
    sw2b = nc.dram_tensor("sw2b", [2 * NSTR, 128, SIT * 256], FP16, kind="ExternalInput")
    idxs = nc.dram_tensor("idxs", [EPC, 128, CAPM // 16], I16, kind="ExternalInput")
    gat = nc.dram_tensor("gat", [EPC, 128, CTM], F32, kind="ExternalInput")
    out = nc.dram_tensor("out", [TOUT, H], FP16, kind="ExternalOutput")

    yL = nc.dram_tensor("yL", [T + 128, H // 2], FP16)
    yR = nc.dram_tensor("yR", [T + 128, H // 2], FP16)
    rsL = nc.dram_tensor("rsL", [TOUT, H // 2], FP16)
    rsR = nc.dram_tensor("rsR", [TOUT, H // 2], FP16)
    warm_in = nc.dram_tensor("warm_in", [2048, 512], FP16)
    warm_out = nc.dram_tensor("warm_out", [256, 512], FP16)
    hb_out = [nc.dram_tensor(f"hb_out{e}", [32, 512], FP16) for e in range(EPC)]

    grp = [list(range(NCORES))]

    with tile.TileContext(nc) as tc:
        with (
            tc.tile_pool(name="const", bufs=1) as const,
            tc.tile_pool(name="persist", bufs=1) as persist,
            tc.tile_pool(name="xgtp", bufs=2) as xgtp,
            tc.tile_pool(name="xgtp1", bufs=2) as xgtp1,
            tc.tile_pool(name="gp", bufs=2) as gp,
            tc.tile_pool(name="w13p", bufs=2) as w13p,
            tc.tile_pool(name="w2p", bufs=2) as w2p,
            tc.tile_pool(name="ybp", bufs=2) as ybp,
            tc.tile_pool(name="s13p", bufs=2) as s13p,
            tc.tile_pool(name="s2p", bufs=2) as s2p,
            tc.tile_pool(name="small", bufs=2) as small,
            tc.tile_pool(name="rsp", bufs=1) as rsp,
            tc.tile_pool(name="psum", bufs=2, space="PSUM") as psum,
        ):
            # ---------------- constants ------------------------------------
            isc = const.tile([128, EPC, CAPM // 16], I16)
            nc.sync.dma_start(isc[:], idxs.rearrange("e p c -> p e c"))
            ga_sb = const.tile([128, EPC, CTM], F32)
            nc.sync.dma_start(ga_sb[:], gat.rearrange("e p c -> p e c"))
            zt = const.tile([128, 1024], FP16)
            nc.vector.memset(zt[:], 0.0)

            gs = persist.tile([128, SIT, TOUT], FP16)
            xtc_sb = persist.tile([128, KT, TOUT], FP16)

            # ---------------- routed experts -------------------------------
            for e in range(EPC):
                CH = CHUNKS[e]
                CAP = CAPS[e]
                CT = CAP // 128
                # token activations pre-gathered+transposed on the host:
                # plain contiguous loads on the fast hwdge queues (the
                # device-side transposed gather was a ~100us startup
                # staircase and hogged the single swdge queue)
                xgta = xgtp.tile([128, KT // 2, CAPM], FP16, tag="xgta")
                xgtb = xgtp1.tile([128, KT // 2, CAPM], FP16, tag="xgtb")
                xgs = xgd[e].rearrange("p (k c) -> p k c", c=CAPM)
                nc.sync.dma_start(xgta[:], xgs[:, :KT // 2, :])
                nc.scalar.dma_start(xgtb[:], xgs[:, KT // 2:, :])
                if e == 0:
                    # warmup collective: after the first gather so it doesn't
                    # delay it (transpose DMAs serialize with collectives).
                    # warm_in is uninitialized garbage - result is discarded.
                    nc.gpsimd.collective_compute(
                        "ReduceScatter", OP.add, replica_groups=grp,
                        ins=[warm_in[:]], outs=[warm_out[:]])
                g = gp.tile([128, MT, CAPM], FP16, tag="g")
                for m in range(MT):
                    w13ta = w13p.tile([128, KT // 2, 256], FP16, tag="w13ta")
                    w13tb = w13p.tile([128, KT // 2, 256], FP16, tag="w13tb")
                    w13s = w13[e, m].rearrange("p (k c) -> p k c", c=256)
                    nc.sync.dma_start(w13ta[:], w13s[:, :KT // 2, :])
                    nc.scalar.dma_start(w13tb[:], w13s[:, KT // 2:, :])
                    c0 = 0
                    for ci, cw in enumerate(CH):
                        p1 = psum.tile([128, 512], F32, tag="p1")
                        p3 = psum.tile([128, 512], F32, tag="p3")
                        for k in range(KT):
                            wh = w13ta if k < KT // 2 else w13tb
                            xh = xgta if k < KT // 2 else xgtb
                            nc.tensor.matmul(p1[:, :cw], wh[:, k % (KT // 2), :128],
                                             xh[:, k % (KT // 2), c0:c0 + cw],
                                             start=(k == 0), stop=(k == KT - 1))
                        for k in range(KT):
                            wh = w13ta if k < KT // 2 else w13tb
                            xh = xgta if k < KT // 2 else xgtb
                            nc.tensor.matmul(p3[:, :cw], wh[:, k % (KT // 2), 128:],
                                             xh[:, k % (KT // 2), c0:c0 + cw],
                                             start=(k == 0), stop=(k == KT - 1))
                        nc.scalar.activation(g[:, m, c0:c0 + cw], p1[:, :cw], act)
                        nc.vector.tensor_tensor(g[:, m, c0:c0 + cw],
                                                g[:, m, c0:c0 + cw],
                                                p3[:, :cw], OP.mult)
                        c0 += cw
                if e == 0:
                    # zero y from the SBUF memset tile: write-only HBM
                    # traffic, emitted after e0's w13 kicks so the first
                    # weight loads aren't stuck behind 16.8MB of zeros.
                    # (A variant putting yR zeros on the swdge queue was
                    # slower: it delays the scatter drain by ~180us.)
                    for r in range(0, T, 128):
                        nc.sync.dma_start(yL[r:r + 128, :], zt[:])
                        nc.scalar.dma_start(yR[r:r + 128, :], zt[:])
                if e == 2:
                    # shared-expert input tokens (needed only at the end;
                    # emitted here to keep it off the startup-critical queues)
                    nc.scalar.dma_start(
                        xtc_sb[:], xTc.rearrange("p (k t) -> p k t", t=TOUT))
                if e < EPC - 1:
                    # zero-dependency heartbeat collective (garbage slice of
                    # warm_in, result discarded): keeps the CC fabric clocked
                    # up through the routed phase (cold-fabric RS runs
                    # ~10-30x slower). No data deps -> Tile cannot entangle
                    # it with compute streams; it launches when the gpsimd
                    # stream reaches it, naturally spaced per expert. None
                    # for the last expert - it would delay the big RS.
                    nc.gpsimd.collective_compute(
                        "ReduceScatter", OP.add, replica_groups=grp,
                        ins=[warm_in[256 * (e + 1):256 * (e + 2), :]],
                        outs=[hb_out[e][:]])
                for q in range(4):
                    w2ta = w2p.tile([128, MT // 2, 512], FP16, tag="w2ta")
                    w2tb = w2p.tile([128, MT // 2, 512], FP16, tag="w2tb")
                    w2s = w2b[e, q].rearrange("p (k c) -> p k c", c=512)
                    nc.sync.dma_start(w2ta[:], w2s[:, :MT // 2, :])
                    nc.scalar.dma_start(w2tb[:], w2s[:, MT // 2:, :])
                    yb = ybp.tile([128, CTM, 512], FP16, tag="yb")
                    for ct in range(CT):
                        p4a = psum.tile([128, 512], F32, tag="p4a")
                        for k2 in range(MT):
                            w2h = w2ta if k2 < MT // 2 else w2tb
                            nc.tensor.matmul(p4a[:], g[:, k2, ct * 128:(ct + 1) * 128],
                                             w2h[:, k2 % (MT // 2), :],
                                             start=(k2 == 0), stop=(k2 == MT - 1))
                        nc.vector.tensor_tensor(
                            yb[:, ct, :], p4a[:],
                            ga_sb[:, e, ct:ct + 1].to_broadcast([128, 512]),
                            OP.mult)
                    # one scatter-add per 512-col block (pad entries -> row
                    # T); cols 0:1024 -> yL, 1024:2048 -> yR so RS_L can
                    # launch as soon as every expert's q=0,1 scatters land
                    ydst = yL if q < 2 else yR
                    nc.gpsimd.dma_scatter_add(
                        ydst[:, (q % 2) * 512:(q % 2 + 1) * 512], yb[:, :CT, :],
                        isc[:, e, :CAP // 16], CAP, CAP, 512, elem_step=H // 2)

            # ---------------- shared experts (own rows) --------------------
            # gs is emitted BEFORE the RS issue: program order guarantees no
            # engine stream has an RS-gated entry ahead of the s13 loads, so
            # the gs compute overlaps the collective.
            for sm in range(SIT):
                s13a = s13p.tile([128, KT // 2, 256], FP16, tag="s13a")
                s13b = s13p.tile([128, KT // 2, 256], FP16, tag="s13b")
                s13s = sw13[sm].rearrange("p (k c) -> p k c", c=256)
                nc.sync.dma_start(s13a[:], s13s[:, :KT // 2, :])
                nc.scalar.dma_start(s13b[:], s13s[:, KT // 2:, :])
                p1 = psum.tile([128, 512], F32, tag="p1")
                p3 = psum.tile([128, 512], F32, tag="p3")
                for k in range(KT):
                    sh = s13a if k < KT // 2 else s13b
                    nc.tensor.matmul(p1[:, :TOUT], sh[:, k % (KT // 2), :128],
                                     xtc_sb[:, k, :],
                                     start=(k == 0), stop=(k == KT - 1))
                for k in range(KT):
                    sh = s13a if k < KT // 2 else s13b
                    nc.tensor.matmul(p3[:, :TOUT], sh[:, k % (KT // 2), 128:],
                                     xtc_sb[:, k, :],
                                     start=(k == 0), stop=(k == KT - 1))
                nc.scalar.activation(gs[:, sm, :], p1[:, :TOUT], act)
                nc.vector.tensor_tensor(gs[:, sm, :], gs[:, sm, :], p3[:, :TOUT],
                                        OP.mult)

            nc.gpsimd.collective_compute(
                "ReduceScatter", OP.add, replica_groups=grp,
                ins=[yL[0:T, :]], outs=[rsL[:]])
            nc.gpsimd.collective_compute(
                "ReduceScatter", OP.add, replica_groups=grp,
                ins=[yR[0:T, :]], outs=[rsR[:]])
            # whole rs halves -> SBUF via the (now idle) swdge queue: two big
            # RS-gated loads there instead of 64 small ones on sync, which
            # wedged the sync queue for ~100us behind the collectives
            rsl_sb = rsp.tile([128, TS, H // 2], FP16, tag="rsb")
            nc.gpsimd.dma_start(rsl_sb[:], rsL.rearrange("(t p) w -> p t w", p=128))
            rsr_sb = rsp.tile([128, TS, H // 2], FP16, tag="rsb")
            nc.gpsimd.dma_start(rsr_sb[:], rsR.rearrange("(t p) w -> p t w", p=128))

            # shared out per 256-col strip + combine with rs.
            # Queue discipline: s2 loads on scalar ONLY; rs reads + out
            # stores on sync ONLY (sync carries nothing compute-path here,
            # so RS-gated kicks can't stall the shared weight stream).
            for s in range(2 * NSTR):
                s2a = s2p.tile([128, SIT // 2, 256], FP16, tag="s2a")
                s2b = s2p.tile([128, SIT // 2, 256], FP16, tag="s2b")
                s2s = sw2b[s].rearrange("p (k c) -> p k c", c=256)
                nc.sync.dma_start(s2a[:], s2s[:, :SIT // 2, :])
                nc.scalar.dma_start(s2b[:], s2s[:, SIT // 2:, :])
                rsh_sb = rsl_sb if s < NSTR else rsr_sb
                sc0 = (s % NSTR) * 256
                for ts in range(TS):
                    po = psum.tile([128, 256], F32, tag="p4a")
                    for k2 in range(SIT):
                        s2h = s2a if k2 < SIT // 2 else s2b
                        nc.tensor.matmul(po[:], gs[:, k2, ts * 128:(ts + 1) * 128],
                                         s2h[:, k2 % (SIT // 2), :],
                                         start=(k2 == 0), stop=(k2 == SIT - 1))
                    ott = small.tile([128, 256], FP16, tag="ott")
                    nc.vector.tensor_tensor(ott[:], po[:],
                                            rsh_sb[:, ts, sc0:sc0 + 256], OP.add)
                    nc.sync.dma_start(
                        out[ts * 128:(ts + 1) * 128, s * 256:(s + 1) * 256],
                        ott[:])

    if compile_:
        nc.compile()
    else:
        nc.insert_library_loads()
    return nc


def host_prep(hidden_states, gate_weight, w1, w2, w3, sw1, sw2, sw3):
    B, S, H = hidden_states.shape
    T = B * S
    E, I = w1.shape[0], w1.shape[1]
    SI = sw1.shape[0]
    EPC = E // NCORES
    KT, MT, SIT = H // 128, I // 128, SI // 128
    NSTR = H // 512
    TOUT = T // NCORES

    x = np.ascontiguousarray(hidden_states.reshape(T, H), dtype=np.float32)
    combine = _routing(x, gate_weight.astype(np.float32))
    tok_lists = [np.nonzero(combine[:, e])[0] for e in range(E)]
    counts = np.array([len(t) for t in tok_lists])

    # rank-matched expert assignment: sort by count desc; slot j holds ranks
    # [8j, 8j+8); core c gets order[8j + c]. Slot capacity covers its max.
    order = np.argsort(-counts, kind="stable")
    CAPS = [max(256, int(np.ceil(counts[order[8 * j]] / 128) * 128))
            for j in range(EPC)]
    CAPM = max(CAPS)
    CTM = CAPM // 128

    x16 = x.astype(np.float16)
    xT = x.T  # [H, T] view

    s1 = sw1.T.reshape(KT, 128, SIT, 128).transpose(2, 1, 0, 3)
    s3 = sw3.T.reshape(KT, 128, SIT, 128).transpose(2, 1, 0, 3)
    sw13 = np.ascontiguousarray(
        np.concatenate([s1, s3], axis=-1).reshape(SIT, 128, -1), dtype=np.float16)
    sw2b = np.ascontiguousarray(
        sw2.T.reshape(SIT, 128, 2 * NSTR, 256).transpose(2, 1, 0, 3)
        .reshape(2 * NSTR, 128, -1), dtype=np.float16)

    xt16 = np.ascontiguousarray(xT, dtype=np.float16)  # [H, T]
    in_maps = []
    for c in range(NCORES):
        els = [int(order[8 * j + c]) for j in range(EPC)]
        xgd = np.zeros((EPC, 128, KT * CAPM), np.float16)
        idxs = np.zeros((EPC, 128, CAPM // 16), np.int16)
        gatv = np.zeros((EPC, 128, CTM), np.float32)
        for j, e in enumerate(els):
            CAP = CAPS[j]
            toks = tok_lists[e]
            n = len(toks)
            # pre-gathered transposed tokens: xgd[j][p, k*CAPM+c] =
            # x16[toks[c], k*128+p] (pads -> 0; their gate weight is 0)
            sel = np.zeros((H, CAPM), np.float16)
            sel[:, :n] = xt16[:, toks]
            xgd[j] = (sel.reshape(KT, 128, CAPM).transpose(1, 0, 2)
                      .reshape(128, -1))
            # scatter: valid entries keep their token row, pads -> row T
            sc = np.full(CAP, T, np.int16)
            sc[:n] = toks
            idxs[j, :, :CAP // 16] = _wrap16(sc)
            gv = np.zeros(CAP, np.float32)
            gv[:n] = combine[toks, e]
            gatv[j, :, :CAP // 128] = gv.reshape(-1, 128).T
        w13c = np.empty((EPC, MT, 128, KT * 256), np.float16)
        w2c = np.empty((EPC, 4, 128, MT * 512), np.float16)
        for j, e in enumerate(els):
            a1 = w1[e].T.reshape(KT, 128, MT, 128).transpose(2, 1, 0, 3)
            a3 = w3[e].T.reshape(KT, 128, MT, 128).transpose(2, 1, 0, 3)
            w13c[j] = np.concatenate([a1, a3], axis=-1).reshape(MT, 128, -1)
            w2c[j] = (w2[e].T.reshape(MT, 128, 4, 512)
                      .transpose(2, 1, 0, 3).reshape(4, 128, -1))
        own_rows = np.arange(c * TOUT, (c + 1) * TOUT)
        xTc = np.ascontiguousarray(
            xT[:, own_rows].reshape(KT, 128, len(own_rows))
            .transpose(1, 0, 2).reshape(128, -1), dtype=np.float16)
        in_maps.append({
            "xgd": xgd, "xTc": xTc,
            "w13": w13c, "w2b": w2c,
            "sw13": sw13, "sw2b": sw2b,
            "idxs": idxs,
            "gat": gatv,
        })
    cfg = dict(T=T, H=H, I=I, EPC=EPC, CAPS=CAPS, SI=SI)
    return in_maps, cfg


def kernel(**inputs):
    inputs = {k: np.asarray(v) for k, v in inputs.items()}
    hs = inputs["hidden_states"]
    B, S, H = hs.shape
    in_maps, cfg = host_prep(
        hs, inputs["gate_weight"], inputs["w1"], inputs["w2"], inputs["w3"],
        inputs["sw1"], inputs["sw2"], inputs["sw3"])
    nc = build_kernel(**cfg)
    res = run_bass_kernel_spmd(nc, in_maps, list(range(NCORES)))
    T = B * S
    TOUT = T // NCORES
    y = np.empty((T, H), np.float32)
    for c in range(NCORES):
        y[c * TOUT:(c + 1) * TOUT] = res.results[c]["out"]
    return y.reshape(B, S, H).astype(np.float32)


if __name__ == "__main__":
    pass


# revision 16
# speedup vs baseline: 1.0174x; 1.0058x over previous
"""DeepSeekV2 MoE layer on 8 trn2 NeuronCores (expert-parallel).

Strategy (v8):
  - Host: gate softmax + group-limited top-k routing -> per-expert sorted token
    lists and combine weights (control data only; all heavy FLOPs on device).
  - Experts are rank-matched to (core, slot): sort by token count desc, slot j
    holds ranks [8j, 8j+8) so slot capacity = count of its largest expert.
    This trims ~7-10% of the padded matmul rows vs one global CAP.
  - Device (SPMD over 8 cores, 4 expert slots each):
      Routed phase: per slot, transposed dma_gathers of its CAP_j tokens in
      chunks [256, 512, ...] (small first chunk -> first matmul starts early)
      on the single Tile-managed swdge queue (multi-queue swdge is racy: sem
      assignment is queue-unaware); mm1/mm3 (fp16) -> silu*mul -> mm2 ->
      scale by combine weight; one scatter-add per (expert, 512-col block)
      into the dense y[T+128, H] (pad entries -> row T).
      Weight loads (w13/w2) are split in half across the two hwdge queues
      (sync + scalar) via separate half-tiles - a single queue can't sustain
      the stream and stalls the PE; two DMAs into one tile is a race (Tile
      dep-tracking is tile-granular).
      y zeroing is 32 SBUF->DRAM writes from a memset tile emitted after
      expert 0's mm13 issue (write-only; off the early weight-load window).
      A 2MB warmup ReduceScatter (garbage in, discarded out) after the first
      gather + small per-expert heartbeat RS for e=0..2 keep the CC fabric
      clocked up: collectives here cost ~75-100us nearly independent of size,
      and a cold fabric ran an 0.25MB RS at ~2.5GB/s.
      ONE ReduceScatter over y[0:T] (not two over row-halves: each extra
      collective pays the ~75us fixed cost) -> rs[512, H]; core c's own
      tokens are the contiguous block [512c, 512c+512).
      Shared experts (full SI) for this core's own 512 rows overlap the RS;
      strict queue discipline in the shared phase: s13/s2 loads on scalar
      ONLY, rs reads + out stores on sync ONLY (an RS-gated kick ahead of a
      compute load on an in-order queue stalls the PE for the whole RS).
  - Host: reassemble contiguous 512-row blocks -> [B, S, H].
"""
import sys

import numpy as np

sys.path.insert(0, "/opt/trn_rl_repo")

import concourse.bass as bass
import concourse.mybir as mybir
import concourse.tile as tile
from concourse import bacc
from concourse.bass_utils import run_bass_kernel_spmd

F32 = mybir.dt.float32
FP16 = mybir.dt.float16
I16 = mybir.dt.int16
AF = mybir.ActivationFunctionType
OP = mybir.AluOpType

N_GROUP, TOPK_GROUP, TOP_K = 8, 3, 6
NCORES = 8


def _routing(x, gate_w):
    T, E = x.shape[0], gate_w.shape[0]
    logits = (x @ gate_w.T).astype(np.float64)
    e = np.exp(logits - logits.max(-1, keepdims=True))
    scores = e / e.sum(-1, keepdims=True)
    per_group = E // N_GROUP
    group_scores = scores.reshape(T, N_GROUP, per_group).max(-1)
    order = np.argsort(-group_scores, axis=-1, kind="stable")
    group_mask = np.zeros((T, N_GROUP), bool)
    np.put_along_axis(group_mask, order[:, :TOPK_GROUP], True, axis=1)
    tmp = np.where(np.repeat(group_mask, per_group, axis=1), scores, 0.0)
    order_e = np.argsort(-tmp, axis=-1, kind="stable")
    topk_idx = order_e[:, :TOP_K]
    topk_w = np.take_along_axis(tmp, topk_idx, axis=1)
    topk_w = topk_w / (topk_w.sum(-1, keepdims=True) + 1e-20)
    combine = np.zeros((T, E), np.float32)
    np.put_along_axis(combine, topk_idx, topk_w.astype(np.float32), axis=1)
    return combine


def _wrap16(a):
    """[n] int16 -> [128, n//16] index layout for dma_gather/scatter."""
    return np.tile(a.reshape(-1, 16).T, (8, 1))


def _chunks(cap):
    """psum-chain column widths: [512, ..., remainder]."""
    out = []
    rem = cap
    while rem >= 512:
        out.append(512)
        rem -= 512
    if rem:
        out.append(rem)
    return out


def build_kernel(T, H, I, EPC, CAPS, SI, act=AF.Silu, compile_=True):
    KT = H // 128          # contraction tiles over H
    MT = I // 128          # I tiles
    SIT = SI // 128
    TOUT = T // NCORES     # own output rows
    TS = TOUT // 128
    NSTR = H // 512
    CAPM = max(CAPS)
    CTM = CAPM // 128
    CHUNKS = [_chunks(c) for c in CAPS]

    nc = bacc.Bacc("TRN2")
    xgd = nc.dram_tensor("xgd", [EPC, 128, KT * CAPM], FP16, kind="ExternalInput")
    xTc = nc.dram_tensor("xTc", [128, KT * TOUT], FP16, kind="ExternalInput")
    w13 = nc.dram_tensor("w13", [EPC, MT, 128, KT * 256], FP16, kind="ExternalInput")
    w2b = nc.dram_tensor("w2b", [EPC, 4, 128, MT * 512], FP16, kind="ExternalInput")
    sw13 = nc.dram_tensor("sw13", [SIT, 128, KT * 256], FP16, kind="ExternalInput")
    sw2b = nc.dram_tensor("sw2b", [2 * NSTR, 128, SIT * 256], FP16, kind="ExternalInput")
    idxs = nc.dram_tensor("idxs", [EPC, 128, CAPM // 16], I16, kind="ExternalInput")
    gat = nc.dram_tensor("gat", [EPC, 128, CTM], F32, kind="ExternalInput")
    out = nc.dram_tensor("out", [TOUT, H], FP16, kind="ExternalOutput")

    yL = nc.dram_tensor("yL", [T + 128, H // 2], FP16)
    yR = nc.dram_tensor("yR", [T + 128, H // 2], FP16)
    rsL = nc.dram_tensor("rsL", [TOUT, H // 2], FP16)
    rsR = nc.dram_tensor("rsR", [TOUT, H // 2], FP16)
    warm_in = nc.dram_tensor("warm_in", [2048, 512], FP16)
    warm_out = nc.dram_tensor("warm_out", [256, 512], FP16)
    hb_out = [nc.dram_tensor(f"hb_out{e}", [32, 512], FP16) for e in range(EPC)]

    grp = [list(range(NCORES))]

    with tile.TileContext(nc) as tc:
        with (
            tc.tile_pool(name="const", bufs=1) as const,
            tc.tile_pool(name="persist", bufs=1) as persist,
            tc.tile_pool(name="xgtp", bufs=2) as xgtp,
            tc.tile_pool(name="xgtp1", bufs=2) as xgtp1,
            tc.tile_pool(name="gp", bufs=2) as gp,
            tc.tile_pool(name="w13p", bufs=2) as w13p,
            tc.tile_pool(name="w2p", bufs=2) as w2p,
            tc.tile_pool(name="ybp", bufs=2) as ybp,
            tc.tile_pool(name="s13p", bufs=2) as s13p,
            tc.tile_pool(name="s2p", bufs=2) as s2p,
            tc.tile_pool(name="small", bufs=2) as small,
            tc.tile_pool(name="rsp", bufs=1) as rsp,
            tc.tile_pool(name="psum", bufs=2, space="PSUM") as psum,
        ):
            # ---------------- constants ------------------------------------
            isc = const.tile([128, EPC, CAPM // 16], I16)
            nc.sync.dma_start(isc[:], idxs.rearrange("e p c -> p e c"))
            ga_sb = const.tile([128, EPC, CTM], F32)
            nc.sync.dma_start(ga_sb[:], gat.rearrange("e p c -> p e c"))
            zt = const.tile([128, 1024], FP16)
            nc.vector.memset(zt[:], 0.0)

            gs = persist.tile([128, SIT, TOUT], FP16)
            xtc_sb = persist.tile([128, KT, TOUT], FP16)

            # ---------------- routed experts -------------------------------
            for e in range(EPC):
                CH = CHUNKS[e]
                CAP = CAPS[e]
                CT = CAP // 128
                # token activations pre-gathered+transposed on the host:
                # plain contiguous loads on the fast hwdge queues (the
                # device-side transposed gather was a ~100us startup
                # staircase and hogged the single swdge queue)
                xgta = xgtp.tile([128, KT // 2, CAPM], FP16, tag="xgta")
                xgtb = xgtp1.tile([128, KT // 2, CAPM], FP16, tag="xgtb")
                xgs = xgd[e].rearrange("p (k c) -> p k c", c=CAPM)
                nc.sync.dma_start(xgta[:], xgs[:, :KT // 2, :])
                nc.scalar.dma_start(xgtb[:], xgs[:, KT // 2:, :])
                if e == 0:
                    # warmup collective: after the first gather so it doesn't
                    # delay it (transpose DMAs serialize with collectives).
                    # warm_in is uninitialized garbage - result is discarded.
                    nc.gpsimd.collective_compute(
                        "ReduceScatter", OP.add, replica_groups=grp,
                        ins=[warm_in[:]], outs=[warm_out[:]])
                g = gp.tile([128, MT, CAPM], FP16, tag="g")
                for m in range(MT):
                    w13ta = w13p.tile([128, KT // 2, 256], FP16, tag="w13ta")
                    w13tb = w13p.tile([128, KT // 2, 256], FP16, tag="w13tb")
                    w13s = w13[e, m].rearrange("p (k c) -> p k c", c=256)
                    nc.sync.dma_start(w13ta[:], w13s[:, :KT // 2, :])
                    nc.scalar.dma_start(w13tb[:], w13s[:, KT // 2:, :])
                    c0 = 0
                    for ci, cw in enumerate(CH):
                        p1 = psum.tile([128, 512], F32, tag="p1")
                        p3 = psum.tile([128, 512], F32, tag="p3")
                        for k in range(KT):
                            wh = w13ta if k < KT // 2 else w13tb
                            xh = xgta if k < KT // 2 else xgtb
                            nc.tensor.matmul(p1[:, :cw], wh[:, k % (KT // 2), :128],
                                             xh[:, k % (KT // 2), c0:c0 + cw],
                                             start=(k == 0), stop=(k == KT - 1))
                        for k in range(KT):
                            wh = w13ta if k < KT // 2 else w13tb
                            xh = xgta if k < KT // 2 else xgtb
                            nc.tensor.matmul(p3[:, :cw], wh[:, k % (KT // 2), 128:],
                                             xh[:, k % (KT // 2), c0:c0 + cw],
                                             start=(k == 0), stop=(k == KT - 1))
                        nc.scalar.activation(g[:, m, c0:c0 + cw], p1[:, :cw], act)
                        nc.vector.tensor_tensor(g[:, m, c0:c0 + cw],
                                                g[:, m, c0:c0 + cw],
                                                p3[:, :cw], OP.mult)
                        c0 += cw
                if e == 0:
                    # zero y from the SBUF memset tile: write-only HBM
                    # traffic, emitted after e0's w13 kicks so the first
                    # weight loads aren't stuck behind 16.8MB of zeros.
                    # (A variant putting yR zeros on the swdge queue was
                    # slower: it delays the scatter drain by ~180us.)
                    for r in range(0, T, 128):
                        nc.sync.dma_start(yL[r:r + 128, :], zt[:])
                        nc.scalar.dma_start(yR[r:r + 128, :], zt[:])
                if e == 2:
                    # shared-expert input tokens (needed only at the end;
                    # emitted here to keep it off the startup-critical queues)
                    nc.scalar.dma_start(
                        xtc_sb[:], xTc.rearrange("p (k t) -> p k t", t=TOUT))
                if e == EPC - 1:
                    # prefetch the first two shared-expert weight tiles so
                    # the gs phase starts without a load-latency gap
                    s13_pre = []
                    for sm in range(2):
                        pa = s13p.tile([128, KT // 2, 256], FP16, tag="s13a")
                        pb = s13p.tile([128, KT // 2, 256], FP16, tag="s13b")
                        s13s = sw13[sm].rearrange("p (k c) -> p k c", c=256)
                        nc.sync.dma_start(pa[:], s13s[:, :KT // 2, :])
                        nc.scalar.dma_start(pb[:], s13s[:, KT // 2:, :])
                        s13_pre.append((pa, pb))
                if e < EPC - 1:
                    # zero-dependency heartbeat collective (garbage slice of
                    # warm_in, result discarded): keeps the CC fabric clocked
                    # up through the routed phase (cold-fabric RS runs
                    # ~10-30x slower). No data deps -> Tile cannot entangle
                    # it with compute streams; it launches when the gpsimd
                    # stream reaches it, naturally spaced per expert. None
                    # for the last expert - it would delay the big RS.
                    nc.gpsimd.collective_compute(
                        "ReduceScatter", OP.add, replica_groups=grp,
                        ins=[warm_in[256 * (e + 1):256 * (e + 2), :]],
                        outs=[hb_out[e][:]])
                for q in range(4):
                    w2ta = w2p.tile([128, MT // 2, 512], FP16, tag="w2ta")
                    w2tb = w2p.tile([128, MT // 2, 512], FP16, tag="w2tb")
                    w2s = w2b[e, q].rearrange("p (k c) -> p k c", c=512)
                    nc.sync.dma_start(w2ta[:], w2s[:, :MT // 2, :])
                    nc.scalar.dma_start(w2tb[:], w2s[:, MT // 2:, :])
                    yb = ybp.tile([128, CTM, 512], FP16, tag="yb")
                    for ct in range(CT):
                        p4a = psum.tile([128, 512], F32, tag="p4a")
                        for k2 in range(MT):
                            w2h = w2ta if k2 < MT // 2 else w2tb
                            nc.tensor.matmul(p4a[:], g[:, k2, ct * 128:(ct + 1) * 128],
                                             w2h[:, k2 % (MT // 2), :],
                                             start=(k2 == 0), stop=(k2 == MT - 1))
                        nc.vector.tensor_tensor(
                            yb[:, ct, :], p4a[:],
                            ga_sb[:, e, ct:ct + 1].to_broadcast([128, 512]),
                            OP.mult)
                    # one scatter-add per 512-col block (pad entries -> row
                    # T); cols 0:1024 -> yL, 1024:2048 -> yR so RS_L can
                    # launch as soon as every expert's q=0,1 scatters land
                    ydst = yL if q < 2 else yR
                    nc.gpsimd.dma_scatter_add(
                        ydst[:, (q % 2) * 512:(q % 2 + 1) * 512], yb[:, :CT, :],
                        isc[:, e, :CAP // 16], CAP, CAP, 512, elem_step=H // 2)

            # ---------------- shared experts (own rows) --------------------
            # gs is emitted BEFORE the RS issue: program order guarantees no
            # engine stream has an RS-gated entry ahead of the s13 loads, so
            # the gs compute overlaps the collective.
            for sm in range(SIT):
                if sm < 2:
                    s13a, s13b = s13_pre[sm]
                else:
                    s13a = s13p.tile([128, KT // 2, 256], FP16, tag="s13a")
                    s13b = s13p.tile([128, KT // 2, 256], FP16, tag="s13b")
                    s13s = sw13[sm].rearrange("p (k c) -> p k c", c=256)
                    nc.sync.dma_start(s13a[:], s13s[:, :KT // 2, :])
                    nc.scalar.dma_start(s13b[:], s13s[:, KT // 2:, :])
                p1 = psum.tile([128, 512], F32, tag="p1")
                p3 = psum.tile([128, 512], F32, tag="p3")
                for k in range(KT):
                    sh = s13a if k < KT // 2 else s13b
                    nc.tensor.matmul(p1[:, :TOUT], sh[:, k % (KT // 2), :128],
                                     xtc_sb[:, k, :],
                                     start=(k == 0), stop=(k == KT - 1))
                for k in range(KT):
                    sh = s13a if k < KT // 2 else s13b
                    nc.tensor.matmul(p3[:, :TOUT], sh[:, k % (KT // 2), 128:],
                                     xtc_sb[:, k, :],
                                     start=(k == 0), stop=(k == KT - 1))
                nc.scalar.activation(gs[:, sm, :], p1[:, :TOUT], act)
                nc.vector.tensor_tensor(gs[:, sm, :], gs[:, sm, :], p3[:, :TOUT],
                                        OP.mult)

            nc.gpsimd.collective_compute(
                "ReduceScatter", OP.add, replica_groups=grp,
                ins=[yL[0:T, :]], outs=[rsL[:]])
            nc.gpsimd.collective_compute(
                "ReduceScatter", OP.add, replica_groups=grp,
                ins=[yR[0:T, :]], outs=[rsR[:]])
            # whole rs halves -> SBUF via the (now idle) swdge queue: two big
            # RS-gated loads there instead of 64 small ones on sync, which
            # wedged the sync queue for ~100us behind the collectives
            rsl_sb = rsp.tile([128, TS, H // 2], FP16, tag="rsb")
            nc.gpsimd.dma_start(rsl_sb[:], rsL.rearrange("(t p) w -> p t w", p=128))
            rsr_sb = rsp.tile([128, TS, H // 2], FP16, tag="rsb")
            nc.gpsimd.dma_start(rsr_sb[:], rsR.rearrange("(t p) w -> p t w", p=128))

            # shared out per 256-col strip + combine with rs.
            # Queue discipline: s2 loads on scalar ONLY; rs reads + out
            # stores on sync ONLY (sync carries nothing compute-path here,
            # so RS-gated kicks can't stall the shared weight stream).
            for s in range(2 * NSTR):
                s2a = s2p.tile([128, SIT // 2, 256], FP16, tag="s2a")
                s2b = s2p.tile([128, SIT // 2, 256], FP16, tag="s2b")
                s2s = sw2b[s].rearrange("p (k c) -> p k c", c=256)
                nc.sync.dma_start(s2a[:], s2s[:, :SIT // 2, :])
                nc.scalar.dma_start(s2b[:], s2s[:, SIT // 2:, :])
                rsh_sb = rsl_sb if s < NSTR else rsr_sb
                sc0 = (s % NSTR) * 256
                for ts in range(TS):
                    po = psum.tile([128, 256], F32, tag="p4a")
                    for k2 in range(SIT):
                        s2h = s2a if k2 < SIT // 2 else s2b
                        nc.tensor.matmul(po[:], gs[:, k2, ts * 128:(ts + 1) * 128],
                                         s2h[:, k2 % (SIT // 2), :],
                                         start=(k2 == 0), stop=(k2 == SIT - 1))
                    ott = small.tile([128, 256], FP16, tag="ott")
                    nc.vector.tensor_tensor(ott[:], po[:],
                                            rsh_sb[:, ts, sc0:sc0 + 256], OP.add)
                    nc.sync.dma_start(
                        out[ts * 128:(ts + 1) * 128, s * 256:(s + 1) * 256],
                        ott[:])

    if compile_:
        nc.compile()
    else:
        nc.insert_library_loads()
    return nc


def host_prep(hidden_states, gate_weight, w1, w2, w3, sw1, sw2, sw3):
    B, S, H = hidden_states.shape
    T = B * S
    E, I = w1.shape[0], w1.shape[1]
    SI = sw1.shape[0]
    EPC = E // NCORES
    KT, MT, SIT = H // 128, I // 128, SI // 128
    NSTR = H // 512
    TOUT = T // NCORES

    x = np.ascontiguousarray(hidden_states.reshape(T, H), dtype=np.float32)
    combine = _routing(x, gate_weight.astype(np.float32))
    tok_lists = [np.nonzero(combine[:, e])[0] for e in range(E)]
    counts = np.array([len(t) for t in tok_lists])

    # rank-matched expert assignment: sort by count desc; slot j holds ranks
    # [8j, 8j+8); core c gets order[8j + c]. Slot capacity covers its max.
    order = np.argsort(-counts, kind="stable")
    CAPS = [max(256, int(np.ceil(counts[order[8 * j]] / 128) * 128))
            for j in range(EPC)]
    CAPM = max(CAPS)
    CTM = CAPM // 128

    x16 = x.astype(np.float16)
    xT = x.T  # [H, T] view

    s1 = sw1.T.reshape(KT, 128, SIT, 128).transpose(2, 1, 0, 3)
    s3 = sw3.T.reshape(KT, 128, SIT, 128).transpose(2, 1, 0, 3)
    sw13 = np.ascontiguousarray(
        np.concatenate([s1, s3], axis=-1).reshape(SIT, 128, -1), dtype=np.float16)
    sw2b = np.ascontiguousarray(
        sw2.T.reshape(SIT, 128, 2 * NSTR, 256).transpose(2, 1, 0, 3)
        .reshape(2 * NSTR, 128, -1), dtype=np.float16)

    xt16 = np.ascontiguousarray(xT, dtype=np.float16)  # [H, T]
    in_maps = []
    for c in range(NCORES):
        els = [int(order[8 * j + c]) for j in range(EPC)]
        xgd = np.zeros((EPC, 128, KT * CAPM), np.float16)
        idxs = np.zeros((EPC, 128, CAPM // 16), np.int16)
        gatv = np.zeros((EPC, 128, CTM), np.float32)
        for j, e in enumerate(els):
            CAP = CAPS[j]
            toks = tok_lists[e]
            n = len(toks)
            # pre-gathered transposed tokens: xgd[j][p, k*CAPM+c] =
            # x16[toks[c], k*128+p] (pads -> 0; their gate weight is 0)
            sel = np.zeros((H, CAPM), np.float16)
            sel[:, :n] = xt16[:, toks]
            xgd[j] = (sel.reshape(KT, 128, CAPM).transpose(1, 0, 2)
                      .reshape(128, -1))
            # scatter: valid entries keep their token row, pads -> row T
            sc = np.full(CAP, T, np.int16)
            sc[:n] = toks
            idxs[j, :, :CAP // 16] = _wrap16(sc)
            gv = np.zeros(CAP, np.float32)
            gv[:n] = combine[toks, e]
            gatv[j, :, :CAP // 128] = gv.reshape(-1, 128).T
        w13c = np.empty((EPC, MT, 128, KT * 256), np.float16)
        w2c = np.empty((EPC, 4, 128, MT * 512), np.float16)
        for j, e in enumerate(els):
            a1 = w1[e].T.reshape(KT, 128, MT, 128).transpose(2, 1, 0, 3)
            a3 = w3[e].T.reshape(KT, 128, MT, 128).transpose(2, 1, 0, 3)
            w13c[j] = np.concatenate([a1, a3], axis=-1).reshape(MT, 128, -1)
            w2c[j] = (w2[e].T.reshape(MT, 128, 4, 512)
                      .transpose(2, 1, 0, 3).reshape(4, 128, -1))
        own_rows = np.arange(c * TOUT, (c + 1) * TOUT)
        xTc = np.ascontiguousarray(
            xT[:, own_rows].reshape(KT, 128, len(own_rows))
            .transpose(1, 0, 2).reshape(128, -1), dtype=np.float16)
        in_maps.append({
            "xgd": xgd, "xTc": xTc,
            "w13": w13c, "w2b": w2c,
            "sw13": sw13, "sw2b": sw2b,
            "idxs": idxs,
            "gat": gatv,
        })
    cfg = dict(T=T, H=H, I=I, EPC=EPC, CAPS=CAPS, SI=SI)
    return in_maps, cfg


def kernel(**inputs):
    inputs = {k: np.asarray(v) for k, v in inputs.items()}
    hs = inputs["hidden_states"]
    B, S, H = hs.shape
    in_maps, cfg = host_prep(
        hs, inputs["gate_weight"], inputs["w1"], inputs["w2"], inputs["w3"],
        inputs["sw1"], inputs["sw2"], inputs["sw3"])
    nc = build_kernel(**cfg)
    res = run_bass_kernel_spmd(nc, in_maps, list(range(NCORES)))
    T = B * S
    TOUT = T // NCORES
    y = np.empty((T, H), np.float32)
    for c in range(NCORES):
        y[c * TOUT:(c + 1) * TOUT] = res.results[c]["out"]
    return y.reshape(B, S, H).astype(np.float32)


if __name__ == "__main__":
    pass
